# revision 11
# baseline (speedup 1.0000x reference)
"""AELoss on 8 TRN2 NeuronCores — block dma_gather front end.

The previous front end issued one SWDGE indirect DMA per 128 gathered
tags (9 serialized instructions, ~12.4us on GpSimd). InstDMAGatherAnt
generates all descriptors of a gather in ONE instruction (994ns fixed +
0.34ns/descriptor) but requires int16 block indices and >=256B per
descriptor. So: gather the 512B block (128 f32) CONTAINING each visible
joint's tag, with the tag map split into two halves so block indices fit
in 15 bits (two instructions, ~2.4us total SWDGE). The wanted lane is
then extracted with one fused DVE op per slot column:
    (iota == off) * G, free-reduced  ->  g[slot]
Slslot-major g columns reduce to per-person (sum g, sum g^2) through the
host-built slot->person one-hot mask via accumulating fp32r PE matmuls.
Tail (pull + pairwise push field) unchanged from the proven kernel:
mean/pull on DVE+ACT, push via PE transpose, exp(-diff^2), one bf16
matmul row-reduce. Ships per-person (pull_p*valid, valid, push row) =
[120,3]; final 4-scalar-per-image normalization on host.
"""

import numpy as np

B = 32
N = 17 * 256 * 256  # 1114112 flattened tag-map size
P = 30              # max people per image
J = 17              # joints per person
M = 8               # cores
BL = B // M         # images per core = 4
PART = BL * P       # person partitions = 120
E = 128             # gathered block size (f32 elems) = 512B
NBLK = BL * N // E  # 34816 blocks per core
SPLIT = 32768       # group L: block < SPLIT (int16 range), H: the rest

_CACHE = {}


def _build(NL, NH):
    """NL/NH: padded slot counts (multiples of 128) for the two groups."""
    from contextlib import ExitStack

    import concourse.tile as tile
    from concourse import bacc, mybir

    f32 = mybir.dt.float32
    f16 = mybir.dt.float16
    i16 = mybir.dt.int16
    Alu = mybir.AluOpType

    CL, CH = NL // 128, NH // 128
    C = CL + CH
    # cst column layout: o | iota | cnt | sel | ident
    OFF_O = 0
    OFF_IOTA = OFF_O + C
    OFF_CNT = OFF_IOTA + E
    OFF_SEL = OFF_CNT + 1
    OFF_ID = OFF_SEL + BL
    CW = OFF_ID + PART

    nc = bacc.Bacc("TRN2", target_bir_lowering=False, debug=False)

    tags = nc.dram_tensor("tags", [BL * N, 1], f32, kind="ExternalInput")
    idxL = nc.dram_tensor("idxL", [128, NL // 16], i16, kind="ExternalInput")
    idxH = nc.dram_tensor("idxH", [128, NH // 16], i16, kind="ExternalInput")
    cst = nc.dram_tensor("cst", [128, CW], f32, kind="ExternalInput")
    jmh = nc.dram_tensor("jmh", [128, C * PART], f16, kind="ExternalInput")
    out = nc.dram_tensor("out", [PART, 3], f32, kind="ExternalOutput")

    tb = tags[:, :].rearrange("(b e) one -> b (one e)", e=E)  # [NBLK, E]

    with tile.TileContext(nc) as tc:
        with ExitStack() as ctx:
            sb = ctx.enter_context(tc.tile_pool(name="sb", bufs=1))
            ps = ctx.enter_context(tc.tile_pool(name="ps", bufs=1, space="PSUM"))

            idxL_t = sb.tile([128, NL // 16], i16)
            nc.sync.dma_start(out=idxL_t[:], in_=idxL[:, :])
            idxH_t = sb.tile([128, NH // 16], i16)
            nc.sync.dma_start(out=idxH_t[:], in_=idxH[:, :])
            cstt = sb.tile([128, CW], f32)
            nc.scalar.dma_start(out=cstt[:], in_=cst[:, :])
            jmt = sb.tile([128, C * PART], f16)
            nc.scalar.dma_start(out=jmt[:], in_=jmh[:, :])
            o_ap = cstt[:, OFF_O:OFF_O + C]
            iota = cstt[:, OFF_IOTA:OFF_IOTA + E]
            cnt = cstt[0:PART, OFF_CNT:OFF_CNT + 1]
            sel = cstt[0:PART, OFF_SEL:OFF_SEL + BL]
            ident = cstt[0:PART, OFF_ID:OFF_ID + PART]

            GL = sb.tile([128, CL, E], f32)
            nc.gpsimd.dma_gather(
                out_ap=GL[:], in_ap=tb[0:SPLIT, :], idxs_ap=idxL_t[:],
                num_idxs=NL, num_idxs_reg=NL, elem_size=E,
            )
            GH = sb.tile([128, CH, E], f32)
            nc.gpsimd.dma_gather(
                out_ap=GH[:], in_ap=tb[SPLIT:NBLK, :], idxs_ap=idxH_t[:],
                num_idxs=NH, num_idxs_reg=NH, elem_size=E,
            )

            # ---- gather-independent chain (runs during the gathers) ----
            safe_cnt = sb.tile([PART, 1], f32)
            nc.vector.tensor_scalar_max(out=safe_cnt[:], in0=cnt, scalar1=1.0)
            icnt = sb.tile([PART, 1], f32)
            nc.vector.reciprocal(out=icnt[:], in_=safe_cnt[:])
            stacked = sb.tile([PART, 3], f32)  # pull_p*valid | valid | push row
            nc.vector.tensor_scalar(out=stacked[:, 1:2], in0=cnt, scalar1=0.0,
                                    scalar2=None, op0=Alu.is_gt)
            vb = sb.tile([PART, BL], f32)
            nc.vector.tensor_scalar(out=vb[:], in0=sel,
                                    scalar1=stacked[:, 1:2], scalar2=None,
                                    op0=Alu.mult)
            vbb = sb.tile([PART, BL], f16)
            nc.vector.tensor_copy(out=vbb[:], in_=vb[:])

            # ---- extract wanted lane per slot column: (iota==off)*G ----
            gv = sb.tile([128, 2 * C], f32)  # g at col 2c, g^2 at 2c+1
            scr = sb.tile([128, E], f32)
            for c in range(C):
                src = GL[:, c, :] if c < CL else GH[:, c - CL, :]
                nc.vector.scalar_tensor_tensor(
                    out=scr[:], in0=iota, scalar=o_ap[:, c:c + 1], in1=src,
                    op0=Alu.is_equal, op1=Alu.mult,
                    accum_out=gv[:, 2 * c:2 * c + 1])
            gvr = gv[:].rearrange("p (c two) -> p c two", two=2)
            nc.vector.tensor_tensor(out=gvr[:, :, 1:2], in0=gvr[:, :, 0:1],
                                    in1=gvr[:, :, 0:1], op=Alu.mult)
            gvh = sb.tile([128, 2 * C], f16)
            nc.vector.tensor_copy(out=gvh[:], in_=gv[:])

            # ---- per-person (sum g, sum g^2): fp16 accumulating matmuls
            Pacc = ps.tile([PART, 2], f32, space="PSUM")
            for c in range(C):
                nc.tensor.matmul(out=Pacc[:],
                                 lhsT=jmt[:, c * PART:(c + 1) * PART],
                                 rhs=gvh[:, 2 * c:2 * c + 2],
                                 start=(c == 0), stop=(c == C - 1))

            mean = sb.tile([PART, 1], f32)
            nc.vector.tensor_tensor(out=mean[:], in0=Pacc[:, 0:1], in1=icnt[:],
                                    op=Alu.mult)
            mean2 = sb.tile([PART, 1], f32)
            nc.vector.tensor_tensor(out=mean2[:], in0=mean[:], in1=mean[:],
                                    op=Alu.mult)

            # pull on ACT in parallel with the DVE/PE push-field chain
            a2 = sb.tile([PART, 1], f32)
            nc.scalar.activation(out=a2[:], in_=Pacc[:, 1:2],
                                 func=mybir.ActivationFunctionType.Copy,
                                 scale=icnt[:])
            nc.vector.scalar_tensor_tensor(out=stacked[:, 0:1], in0=a2[:],
                                           scalar=mean2[:], in1=stacked[:, 1:2],
                                           op0=Alu.subtract, op1=Alu.mult)

            meanT = ps.tile([PART, PART], f32, space="PSUM")
            nc.tensor.transpose(out=meanT[:],
                                in_=mean[:].to_broadcast([PART, PART]),
                                identity=ident)
            diff = sb.tile([PART, PART], f32)
            nc.vector.tensor_tensor(out=diff[:],
                                    in0=mean[:].to_broadcast([PART, PART]),
                                    in1=meanT[:], op=Alu.subtract)
            sq = sb.tile([PART, PART], f32)
            nc.vector.tensor_tensor(out=sq[:], in0=diff[:], in1=diff[:],
                                    op=Alu.mult)
            pm = sb.tile([PART, PART], f16)
            nc.scalar.activation(out=pm[:], in_=sq[:],
                                 func=mybir.ActivationFunctionType.Exp,
                                 scale=-1.0)

            # push rows: (pm @ vb) * vb, row-summed — pm is symmetric, so
            # lhsT=pm gives sum_r pm[q,r]*valid_r*same_image(r,b)
            pvb = ps.tile([PART, BL], f32, space="PSUM")
            nc.tensor.matmul(out=pvb[:], lhsT=pm[:], rhs=vbb[:],
                             start=True, stop=True)
            t1 = sb.tile([PART, BL], f32)
            nc.vector.scalar_tensor_tensor(out=t1[:], in0=pvb[:], scalar=1.0,
                                           in1=vb[:], op0=Alu.mult,
                                           op1=Alu.mult,
                                           accum_out=stacked[:, 2:3])

            nc.sync.dma_start(out=out[:, :], in_=stacked[:])

    nc.compile()
    return nc


def _get_nc(NL, NH):
    key = (NL, NH)
    if key not in _CACHE:
        _CACHE[key] = _build(NL, NH)
    return _CACHE[key]


def _pack_idx(blocks, NK):
    """Pack the block stream into the SWDGE int16 index tile: k-th index at
    [k%16, k//16], 16-row pattern replicated across all 128 partitions."""
    stream = np.zeros(NK, np.int16)
    stream[:len(blocks)] = blocks
    t16 = np.zeros((16, NK // 16), np.int16)
    k = np.arange(NK)
    t16[k % 16, k // 16] = stream
    return np.ascontiguousarray(np.tile(t16, (8, 1)))


def _round_up(n, m):
    return max(m, (n + m - 1) // m * m)


def _make_in_maps(tags: np.ndarray, joints: np.ndarray):
    tags = np.asarray(tags, dtype=np.float32).reshape(B, N)
    joints = np.asarray(joints, dtype=np.int32)

    sel = np.repeat(np.eye(BL, dtype=np.float32), P, axis=0)       # [120, BL]
    ident = np.eye(PART, dtype=np.float32)                         # [120, 120]
    iota = np.broadcast_to(np.arange(E, dtype=np.float32), (128, E))

    cores = []
    NLmax = NHmax = 128
    for i in range(M):
        sl = joints[i * BL:(i + 1) * BL]  # [BL, P, J, 2]
        vis = sl[..., 1] > 0
        bb, pp, jj = np.nonzero(vis)
        flat = (sl[..., 0][bb, pp, jj].astype(np.int64) + bb * N)
        person = (bb * P + pp).astype(np.int64)
        block = flat // E
        off = (flat % E).astype(np.float32)
        inL = block < SPLIT
        cores.append((vis, person, block, off, inL))
        NLmax = max(NLmax, _round_up(int(inL.sum()), 128))
        NHmax = max(NHmax, _round_up(int((~inL).sum()), 128))

    NL, NH = NLmax, NHmax
    CL, CH = NL // 128, NH // 128
    C = CL + CH

    in_maps = []
    for i in range(M):
        vis, person, block, off, inL = cores[i]
        o = np.full((128, C), -1.0, np.float32)
        jm = np.zeros((128, C * PART), np.float16)
        for grp, base, colbase in ((inL, 0, 0), (~inL, SPLIT, CL)):
            k = np.arange(int(grp.sum()))
            prow, pcol = k % 128, colbase + k // 128
            o[prow, pcol] = off[grp]
            jm[prow, pcol * PART + person[grp]] = 1.0
        idxl = _pack_idx((block[inL] - 0).astype(np.int16), NL)
        idxh = _pack_idx((block[~inL] - SPLIT).astype(np.int16), NH)
        cnt = np.zeros((128, 1), np.float32)
        cnt[:PART, 0] = vis.sum(-1).astype(np.float32).reshape(PART)
        pad = np.zeros((8, BL + PART), np.float32)
        si = np.concatenate([np.concatenate([sel, ident], axis=1), pad], axis=0)
        cst = np.concatenate([o, iota, cnt, si], axis=1)  # [128, CW]
        t = tags[i * BL:(i + 1) * BL].reshape(BL * N, 1)
        in_maps.append({"tags": t, "idxL": idxl, "idxH": idxh,
                        "cst": np.ascontiguousarray(cst),
                        "jmh": np.ascontiguousarray(jm)})
    return in_maps, (NL, NH)


def _finalize(stacked: np.ndarray):
    # stacked: [PART, 3] per-person (pull_p*valid, valid, push row); the
    # per-image reduction + final normalization run on host.
    red = stacked.astype(np.float64).reshape(BL, P, 3).sum(axis=1)
    pull_sum = red[:, 0]
    nt = red[:, 1]
    push_tot = red[:, 2]
    pull = pull_sum / np.maximum(nt, 1.0)
    denom = np.maximum((nt - 1.0) * nt, 1.0)
    push = np.where(nt > 1.0, (push_tot - nt) / denom * 0.5, 0.0)
    return push.astype(np.float32), pull.astype(np.float32)


def _run(tags, joints, trace=False):
    from concourse.bass_utils import run_bass_kernel_spmd

    in_maps, (NL, NH) = _make_in_maps(tags, joints)
    nc = _get_nc(NL, NH)
    res = run_bass_kernel_spmd(
        nc, in_maps, core_ids=list(range(M)), trace=trace,
    )
    push = np.empty(B, np.float32)
    pull = np.empty(B, np.float32)
    for i in range(M):
        p, q = _finalize(np.asarray(res.results[i]["out"]))
        push[i * BL:(i + 1) * BL] = p
        pull[i * BL:(i + 1) * BL] = q
    return (push, pull), res.exec_time_ns


def kernel(tags, joints):
    try:
        (push, pull), _ = _run(tags, joints, trace=False)
    except Exception:
        (push, pull), _ = _run(tags, joints, trace=False)
    return push, pull


# revision 12
# speedup vs baseline: 1.3273x; 1.3273x over previous
"""AELoss on 8 TRN2 NeuronCores — visible-packed gather, fp16 reduction.

Front end keeps the proven layout: visible joints packed into a [128, 9]
slot grid, one indirect DMA per column (the ~1.39us/instruction SWDGE
descriptor pass serializes on GpSimd and dominates; InstDMAGatherAnt was
measured strictly worse: ~2.8us ucode lib load + ~7.6ns/descriptor).
Everything around the gathers is leaner than before:
- joff ships on the Scalar queue, which reaches the kernel body ~0.6us
  before Sync, starting the gather chain earlier.
- Per-person (sum g, sum g^2) accumulate through fp16 matmuls (one
  LDWEIGHTS+MATMUL per column, ~0.4us vs ~1.3us for fp32 LOW_HIGH; the
  slot->person one-hot mask ships as fp16, halving that DMA).
- g and g^2 are squared/cast on DVE per column while later gathers run.
- The pairwise push field folds its mask and row reduction through one
  fp16 PE matmul (pm @ vb, pm is symmetric) and one fused
  multiply+row-sum.
The device ships per-person (pull_p*valid, valid, push row) = [120,3];
the final normalization of 4 scalars per core happens on host.
"""

import numpy as np

B = 32
N = 17 * 256 * 256  # 1114112 flattened tag-map size
P = 30              # max people per image
J = 17              # joints per person
M = 8               # cores
BL = B // M         # images per core = 4
PART = BL * P       # person partitions = 120
C = 9               # packed gather columns (capacity 128*9 = 1152 slots)
CW = 1 + BL + PART  # consts width: cnt | sel | identity

_CACHE = {}


def _build():
    from contextlib import ExitStack

    import concourse.bass as bass
    import concourse.tile as tile
    from concourse import bacc, mybir

    f32 = mybir.dt.float32
    f16 = mybir.dt.float16
    i32 = mybir.dt.int32
    Alu = mybir.AluOpType

    nc = bacc.Bacc("TRN2", target_bir_lowering=False, debug=False)

    tags = nc.dram_tensor("tags", [BL * N, 1], f32, kind="ExternalInput")
    joff = nc.dram_tensor("joff", [128, C], i32, kind="ExternalInput")
    jmh = nc.dram_tensor("jmh", [128, C * PART], f16, kind="ExternalInput")
    cst = nc.dram_tensor("cst", [PART, CW], f32, kind="ExternalInput")
    out = nc.dram_tensor("out", [PART, 3], f32, kind="ExternalOutput")

    with tile.TileContext(nc) as tc:
        with ExitStack() as ctx:
            sb = ctx.enter_context(tc.tile_pool(name="sb", bufs=1))
            ps = ctx.enter_context(tc.tile_pool(name="ps", bufs=1, space="PSUM"))

            # Scalar reaches the body earliest — joff first to launch the
            # gather chain as soon as possible.
            joff_t = sb.tile([128, C], i32)
            nc.scalar.dma_start(out=joff_t[:], in_=joff[:, :])
            jmt = sb.tile([128, C * PART], f16)
            nc.scalar.dma_start(out=jmt[:], in_=jmh[:, :])
            cstt = sb.tile([PART, CW], f32)
            nc.sync.dma_start(out=cstt[:], in_=cst[:, :])
            cnt = cstt[:, 0:1]
            sel = cstt[:, 1:1 + BL]
            ident = cstt[:, 1 + BL:CW]

            # Tf pairs: g at col 2c, g^2 at 2c+1; gvh is the fp16 copy.
            Tf = sb.tile([128, 2 * C], f32)
            for c in range(C):
                nc.gpsimd.indirect_dma_start(
                    out=Tf[:, 2 * c:2 * c + 1],
                    out_offset=None,
                    in_=tags[:, :],
                    in_offset=bass.IndirectOffsetOnAxis(
                        ap=joff_t[:, c:c + 1], axis=0),
                )

            # ---- gather-independent chain (runs during the gathers) ----
            safe_cnt = sb.tile([PART, 1], f32)
            nc.vector.tensor_scalar_max(out=safe_cnt[:], in0=cnt, scalar1=1.0)
            icnt = sb.tile([PART, 1], f32)
            nc.vector.reciprocal(out=icnt[:], in_=safe_cnt[:])
            stacked = sb.tile([PART, 3], f32)  # pull_p*valid | valid | push row
            nc.vector.tensor_scalar(out=stacked[:, 1:2], in0=cnt, scalar1=0.0,
                                    scalar2=None, op0=Alu.is_gt)
            vb = sb.tile([PART, BL], f32)
            nc.vector.tensor_scalar(out=vb[:], in0=sel,
                                    scalar1=stacked[:, 1:2], scalar2=None,
                                    op0=Alu.mult)
            vbb = sb.tile([PART, BL], f16)
            nc.vector.tensor_copy(out=vbb[:], in_=vb[:])

            # per-person (sum g, sum g^2) via accumulating fp16 one-hot
            # matmuls; square + cast chase each gather column on DVE.
            gvh = sb.tile([128, 2 * C], f16)
            Pacc = ps.tile([PART, 2], f32, space="PSUM")
            for c in range(C):
                nc.vector.tensor_tensor(out=Tf[:, 2 * c + 1:2 * c + 2],
                                        in0=Tf[:, 2 * c:2 * c + 1],
                                        in1=Tf[:, 2 * c:2 * c + 1],
                                        op=Alu.mult)
                nc.vector.tensor_copy(out=gvh[:, 2 * c:2 * c + 2],
                                      in_=Tf[:, 2 * c:2 * c + 2])
                nc.tensor.matmul(out=Pacc[:],
                                 lhsT=jmt[:, c * PART:(c + 1) * PART],
                                 rhs=gvh[:, 2 * c:2 * c + 2],
                                 start=(c == 0), stop=(c == C - 1))

            mean = sb.tile([PART, 1], f32)
            nc.vector.tensor_tensor(out=mean[:], in0=Pacc[:, 0:1], in1=icnt[:],
                                    op=Alu.mult)
            mean2 = sb.tile([PART, 1], f32)
            nc.vector.tensor_tensor(out=mean2[:], in0=mean[:], in1=mean[:],
                                    op=Alu.mult)

            # pull on ACT in parallel with the DVE/PE push-field chain
            a2 = sb.tile([PART, 1], f32)
            nc.scalar.activation(out=a2[:], in_=Pacc[:, 1:2],
                                 func=mybir.ActivationFunctionType.Copy,
                                 scale=icnt[:])
            nc.vector.scalar_tensor_tensor(out=stacked[:, 0:1], in0=a2[:],
                                           scalar=mean2[:], in1=stacked[:, 1:2],
                                           op0=Alu.subtract, op1=Alu.mult)

            meanT = ps.tile([PART, PART], f32, space="PSUM")
            nc.tensor.transpose(out=meanT[:],
                                in_=mean[:].to_broadcast([PART, PART]),
                                identity=ident)
            diff = sb.tile([PART, PART], f32)
            nc.vector.tensor_tensor(out=diff[:],
                                    in0=mean[:].to_broadcast([PART, PART]),
                                    in1=meanT[:], op=Alu.subtract)
            sq = sb.tile([PART, PART], f32)
            nc.vector.tensor_tensor(out=sq[:], in0=diff[:], in1=diff[:],
                                    op=Alu.mult)
            pm = sb.tile([PART, PART], f16)
            nc.scalar.activation(out=pm[:], in_=sq[:],
                                 func=mybir.ActivationFunctionType.Exp,
                                 scale=-1.0)

            # push rows: (pm @ vb) * vb, row-summed — pm is symmetric, so
            # lhsT=pm gives sum_r pm[q,r]*valid_r*same_image(r,b)
            pvb = ps.tile([PART, BL], f32, space="PSUM")
            nc.tensor.matmul(out=pvb[:], lhsT=pm[:], rhs=vbb[:],
                             start=True, stop=True)
            t1 = sb.tile([PART, BL], f32)
            nc.vector.scalar_tensor_tensor(out=t1[:], in0=pvb[:], scalar=1.0,
                                           in1=vb[:], op0=Alu.mult,
                                           op1=Alu.mult,
                                           accum_out=stacked[:, 2:3])

            nc.sync.dma_start(out=out[:, :], in_=stacked[:])

    nc.compile()
    return nc


def _get_nc():
    if "nc" not in _CACHE:
        _CACHE["nc"] = _build()
    return _CACHE["nc"]


def _make_in_maps(tags: np.ndarray, joints: np.ndarray):
    tags = np.asarray(tags, dtype=np.float32).reshape(B, N)
    joints = np.asarray(joints, dtype=np.int32)

    sel = np.repeat(np.eye(BL, dtype=np.float32), P, axis=0)       # [120, BL]
    ident = np.eye(PART, dtype=np.float32)                         # [120, 120]

    in_maps = []
    for i in range(M):
        t = tags[i * BL:(i + 1) * BL].reshape(BL * N, 1)
        sl = joints[i * BL:(i + 1) * BL]  # [BL, P, J, 2]
        vis = sl[..., 1] > 0
        bb, pp, jj = np.nonzero(vis)
        n = bb.size
        assert n <= 128 * C, f"visible joints {n} exceed slot capacity {128 * C}"
        tag_idx = (sl[..., 0][bb, pp, jj] + bb * N).astype(np.int32)
        person = (bb * P + pp).astype(np.int32)
        k = np.arange(n)
        prow, pcol = k % 128, k // 128
        joff = np.zeros((128, C), np.int32)
        joff[prow, pcol] = tag_idx
        jm = np.zeros((128, C * PART), np.float16)
        jm[prow, pcol * PART + person] = 1.0
        cnt = vis.sum(-1).astype(np.float32).reshape(PART, 1)
        cst = np.concatenate([cnt, sel, ident], axis=1)  # [120, CW]
        in_maps.append({"tags": t, "joff": joff, "jmh": jm,
                        "cst": np.ascontiguousarray(cst)})
    return in_maps


def _finalize(stacked: np.ndarray):
    # stacked: [PART, 3] per-person (pull_p*valid, valid, push row); the
    # per-image reduction + final normalization run on host.
    red = stacked.astype(np.float64).reshape(BL, P, 3).sum(axis=1)
    pull_sum = red[:, 0]
    nt = red[:, 1]
    push_tot = red[:, 2]
    pull = pull_sum / np.maximum(nt, 1.0)
    denom = np.maximum((nt - 1.0) * nt, 1.0)
    push = np.where(nt > 1.0, (push_tot - nt) / denom * 0.5, 0.0)
    return push.astype(np.float32), pull.astype(np.float32)


def _run(tags, joints, trace=False):
    from concourse.bass_utils import run_bass_kernel_spmd

    nc = _get_nc()
    in_maps = _make_in_maps(tags, joints)
    res = run_bass_kernel_spmd(
        nc, in_maps, core_ids=list(range(M)), trace=trace,
    )
    push = np.empty(B, np.float32)
    pull = np.empty(B, np.float32)
    for i in range(M):
        p, q = _finalize(np.asarray(res.results[i]["out"]))
        push[i * BL:(i + 1) * BL] = p
        pull[i * BL:(i + 1) * BL] = q
    return (push, pull), res.exec_time_ns


def kernel(tags, joints):
    try:
        (push, pull), _ = _run(tags, joints, trace=False)
    except Exception:
        (push, pull), _ = _run(tags, joints, trace=False)
    return push, pull
